# revision 1
# baseline (speedup 1.0000x reference)
"""ConvDVSGestureSNN Trainium2 kernel v2: 8-core data-parallel (16 batch each).

Per core/timestep:
- conv1 as 6x6 stride-2 fp16 matmuls, K=36 rows (e,j,ci) x 2 f-rounds, with
  beta1*v1*(1-spk) folded into the PSUM group via a diag(beta1) matmul.
- LIF1 spikes as complement (is_le) writing replica j2=0; j2=2,4 replicas via
  SBUF->SBUF DMA (Pool/SWDGE path). Reset mask = rep0.
- conv2 over complement reps, K=96 rows (j2,c) x 12 (ey,jp) rounds + diag(beta2).
- fc1 transposed-out: 13 K=128 rounds x 2 halves -> I_fc^T [128u, 16b].
- adaptive LIF on [128, (2h,16b)]; fc_out via spkfc as lhsT (no transpose).
T=50 fully unrolled.
"""
import numpy as np

B_LOC, T = 16, 50
N_FC, N_OUT = 256, 11
EPS = 1e-5
XROW = 1032            # padded per-(t,ci) image row (1024 data + 8)
XT = 2 * XROW          # per-t block
XB = T * XT + 160      # per-batch block, +160 tail pad for shifted reads
VPAD = 3144            # v1/rep free size (3136 + 8 shift pad)


def _sig(z):
    return 1.0 / (1.0 + np.exp(-np.asarray(z, np.float64)))


def _build_nc():
    import concourse.bass as bass
    import concourse.mybir as mybir
    import concourse.tile as tile
    from concourse import bacc

    dt = mybir.dt
    Alu = mybir.AluOpType
    Act = mybir.ActivationFunctionType

    nc = bacc.Bacc("TRN2", target_bir_lowering=False, debug=False)

    xr = nc.dram_tensor("xr", [B_LOC, XB], dt.float16, kind="ExternalInput")
    A1 = nc.dram_tensor("A1", [36, 64], dt.float16, kind="ExternalInput")
    D1 = nc.dram_tensor("D1", [32, 32], dt.float16, kind="ExternalInput")
    B1C = nc.dram_tensor("B1C", [32, 1], dt.float32, kind="ExternalInput")
    A2 = nc.dram_tensor("A2", [96, 12 * 64], dt.float16, kind="ExternalInput")
    D2 = nc.dram_tensor("D2", [64, 64], dt.float16, kind="ExternalInput")
    B2C = nc.dram_tensor("B2C", [64, 1], dt.float32, kind="ExternalInput")
    F1T = nc.dram_tensor("F1T", [128, 26 * 128], dt.float16, kind="ExternalInput")
    ALPH = nc.dram_tensor("ALPH", [128, 2], dt.float32, kind="ExternalInput")
    RHO2 = nc.dram_tensor("RHO2", [128, 2], dt.float32, kind="ExternalInput")
    RHOC2 = nc.dram_tensor("RHOC2", [128, 2], dt.float32, kind="ExternalInput")
    BA2 = nc.dram_tensor("BA2", [128, 2], dt.float32, kind="ExternalInput")
    FO = nc.dram_tensor("FO", [128, 2 * N_OUT], dt.float16, kind="ExternalInput")
    OUT = nc.dram_tensor("out", [B_LOC, N_OUT], dt.float32, kind="ExternalOutput")
    if CFG.get("debug_out"):
        OUTV1 = nc.dram_tensor("outv1", [32, 3136], dt.float32, kind="ExternalOutput")
        OUTV2 = nc.dram_tensor("outv2", [64, 400], dt.float32, kind="ExternalOutput")

    def bcast16(tileap):
        # [128,2] -> [128,2,16] stride-0 broadcast over batch
        return bass.AP(tileap.tensor, tileap.offset, tileap.ap.copy() + [[0, 16]])

    with tile.TileContext(nc) as tc:
        with tc.tile_pool(name="const", bufs=1) as cp, \
             tc.tile_pool(name="state", bufs=1) as st, \
             tc.tile_pool(name="xp", bufs=CFG["xp_bufs"]) as xp, \
             tc.tile_pool(name="rp", bufs=2) as rp, \
             tc.tile_pool(name="sp2", bufs=2) as sp2p, \
             tc.tile_pool(name="wp", bufs=2) as wp, \
             tc.tile_pool(name="ps1", bufs=CFG["ps1_bufs"], space="PSUM") as ps1, \
             tc.tile_pool(name="ps2", bufs=2, space="PSUM") as ps2, \
             tc.tile_pool(name="psf", bufs=1, space="PSUM") as psf, \
             tc.tile_pool(name="pso", bufs=1, space="PSUM") as pso:

            a1 = cp.tile([36, 64], dt.float16)
            d1 = cp.tile([32, 32], dt.float16)
            b1c = cp.tile([32, 1], dt.float32)
            a2 = cp.tile([96, 12 * 64], dt.float16)
            d2 = cp.tile([64, 64], dt.float16)
            b2c = cp.tile([64, 1], dt.float32)
            f1t = cp.tile([128, 26 * 128], dt.float16)
            alph = cp.tile([128, 2], dt.float32)
            rho2 = cp.tile([128, 2], dt.float32)
            rhoc2 = cp.tile([128, 2], dt.float32)
            ba2 = cp.tile([128, 2], dt.float32)
            fo = cp.tile([128, 2 * N_OUT], dt.float16)
            for dst, src in ((a1, A1), (d1, D1), (b1c, B1C), (a2, A2), (d2, D2),
                             (b2c, B2C), (f1t, F1T), (alph, ALPH), (rho2, RHO2),
                             (rhoc2, RHOC2), (ba2, BA2), (fo, FO)):
                nc.sync.dma_start(dst[:], src[:])

            # persistent state
            v1 = st.tile([32, VPAD], dt.float16)     # (16b,14,14)+pad
            v1m = st.tile([32, 3136], dt.float16)    # v1*(1-spk) for diag accum
            v2 = st.tile([64, 400], dt.float16)      # (16b,5,5)
            v2m = st.tile([64, 400], dt.float16)
            vfc = st.tile([128, 32], dt.float32)     # rows u, free (2h,16b)
            afc = st.tile([128, 32], dt.float32)
            spkfc = st.tile([128, 32], dt.float16)
            vo = st.tile([16, N_OUT], dt.float32)
            acc = st.tile([16, N_OUT], dt.float32)
            for z in (v1, v1m, v2m, vfc, afc, spkfc, vo, acc):
                nc.gpsimd.memset(z[:], 0.0)

            def stageA(t):
                # x36 load + conv1 (+beta1*v1m diag) + evacs + LIF1 spikes/reps
                x36 = xp.tile([36, 16 * 1024], dt.float16, tag="x36")
                for e in range(3):
                    for ci in range(2):
                        src = bass.AP(xr[:].tensor, t * XT + ci * XROW + 64 * e,
                                      [[1, 1], [1, 6], [XB, 16], [1, 1024]])
                        dst = x36[e * 12 + ci * 6:e * 12 + ci * 6 + 6, :] \
                            .rearrange("p (b f) -> p b f", b=16, f=1024)
                        eng = nc.gpsimd if (CFG["x36_pool_ci1"] and ci == 1) \
                            else nc.sync
                        eng.dma_start(dst, src)
                yield
                x36v = x36[:].rearrange("p (b y x) -> p b y x", b=16, y=32, x=32)
                reps = rp.tile([96, VPAD], dt.float16, tag="reps")
                nsp = CFG["lif1_split"]
                assert nsp == 8
                for c in range(8):
                    p1 = ps1.tile([32, 392], dt.float32, tag="p1")
                    p1v = p1[:].rearrange("p (b y x) -> p b y x", b=2, y=14, x=14)
                    nc.tensor.matmul(p1v, a1[:, 0:32],
                                     x36v[0:36, 2 * c:2 * c + 2, 0:28:2, 0:27:2],
                                     start=True, stop=False)
                    nc.tensor.matmul(p1v, a1[:, 32:64],
                                     x36v[0:36, 2 * c:2 * c + 2, 1:28:2, 0:27:2],
                                     start=False, stop=False)
                    nc.tensor.matmul(p1[:], d1[:],
                                     v1m[:, c * 392:(c + 1) * 392],
                                     start=False, stop=True)
                    if CFG["evac_dve"] and c % 2 == 1:
                        nc.vector.tensor_scalar(v1[:, c * 392:(c + 1) * 392],
                                                p1[:], b1c[:], None, Alu.add)
                    else:
                        nc.scalar.activation(v1[:, c * 392:(c + 1) * 392], p1[:],
                                             Act.Identity, bias=b1c[:])
                    lo = c * 392
                    hi = lo + 392 + (8 if c == 7 else 0)
                    nc.vector.tensor_scalar(reps[0:32, lo:hi], v1[:, lo:hi],
                                            1.0, None, Alu.is_le)
                    nc.vector.tensor_tensor(v1m[:, lo:lo + 392],
                                            v1[:, lo:lo + 392],
                                            reps[0:32, lo:lo + 392], Alu.mult)
                    yield
                # chunked shifted replicas; conv2 windows only read x<10,
                # so write just those columns (strided fp16 stays in 4x mode)
                for cc in range(8):
                    lo = cc * 392
                    for g, j2, eng in ((1, 2, nc.vector), (2, 4, nc.gpsimd)):
                        dstv = reps[32 * g:32 * g + 32, lo:lo + 392] \
                            .rearrange("p (b y x) -> p b y x",
                                       b=2, y=14, x=14)[:, :, :, 0:10]
                        srcv = v1[:, lo + j2:lo + j2 + 392] \
                            .rearrange("p (b y x) -> p b y x",
                                       b=2, y=14, x=14)[:, :, :, 0:10]
                        eng.tensor_scalar(dstv, srcv, 1.0, None, Alu.is_le)
                    yield
                repsq[t] = reps

            def stageB(t):
                # conv2 (+beta2*v2m diag) + evac + LIF2 spikes
                reps = repsq.pop(t)
                repv = reps[0:96, 0:3136].rearrange("p (b y x) -> p b y x",
                                                    b=16, y=14, x=14)
                p2 = ps2.tile([64, 400], dt.float32, tag="p2")
                p2v = p2[:].rearrange("p (b y x) -> p b y x", b=16, y=5, x=5)
                r = 0
                for ey in range(6):
                    for jp in range(2):
                        nc.tensor.matmul(p2v, a2[:, r * 64:(r + 1) * 64],
                                         repv[0:96, :, ey:ey + 9:2, jp:jp + 9:2],
                                         start=(r == 0), stop=False)
                        r += 1
                        if r % 4 == 0:
                            yield
                nc.tensor.matmul(p2[:], d2[:], v2m[:], start=False, stop=True)
                nc.scalar.activation(v2[:], p2[:], Act.Identity, bias=b2c[:])
                yield
                # spk2 free layout (s13, b): fc1 rounds read contiguous 16-col
                # slices (PE moving port rejects odd-granule strides)
                spk2 = sp2p.tile([128, 16 * 13], dt.float16, tag="spk2")
                nc.gpsimd.memset(spk2[64:128, 192:208], 0.0)
                v2v = v2[:].rearrange("p (b s) -> p b s", b=16, s=25)
                s2v = spk2[:].rearrange("p (s b) -> p b s", s=13, b=16)
                nc.vector.tensor_scalar(s2v[0:64, :, 0:13], v2v[:, :, 0:25:2],
                                        1.0, None, Alu.is_gt)
                nc.vector.tensor_scalar(s2v[64:128, :, 0:12], v2v[:, :, 1:25:2],
                                        1.0, None, Alu.is_gt)
                m2 = wp.tile([64, 400], dt.float16, tag="m2")
                nc.gpsimd.tensor_scalar(m2[:], v2[:], 1.0, None, Alu.is_le)
                nc.gpsimd.tensor_tensor(v2m[:], v2[:], m2[:], Alu.mult)
                yield
                spk2q[t] = spk2

            def stageC(t):
                # fc1 transposed-out + adaptive LIF + fc_out accumulation
                spk2 = spk2q.pop(t)
                pf = []
                for h in range(2):
                    pfh = psf.tile([128, 16], dt.float32, tag=f"pf{h}")
                    for s13 in range(13):
                        rhs = spk2[:, s13 * 16:(s13 + 1) * 16]
                        nc.tensor.matmul(pfh[:], f1t[:, (s13 * 2 + h) * 128:
                                                      (s13 * 2 + h + 1) * 128],
                                         rhs, start=(s13 == 0), stop=(s13 == 12))
                    pf.append(pfh)
                    yield
                afv = afc[:].rearrange("p (h b) -> p h b", h=2, b=16)
                tmpa = wp.tile([128, 32], dt.float32, tag="tmpa")
                tav = tmpa[:].rearrange("p (h b) -> p h b", h=2, b=16)
                sfv = spkfc[:].rearrange("p (h b) -> p h b", h=2, b=16)
                nc.gpsimd.tensor_tensor(tav, sfv, bcast16(rhoc2[:]), Alu.mult)
                nc.gpsimd.tensor_tensor(afv, afv, bcast16(rho2[:]), Alu.mult)
                nc.gpsimd.tensor_tensor(afc[:], afc[:], tmpa[:], Alu.add)
                for h in range(2):
                    nc.vector.scalar_tensor_tensor(
                        vfc[:, h * 16:(h + 1) * 16], vfc[:, h * 16:(h + 1) * 16],
                        alph[:, h:h + 1], pf[h][:], Alu.mult, Alu.add)
                yield
                th = wp.tile([128, 32], dt.float32, tag="th")
                thv = th[:].rearrange("p (h b) -> p h b", h=2, b=16)
                nc.gpsimd.tensor_tensor(thv, afv, bcast16(ba2[:]), Alu.mult)
                nc.gpsimd.tensor_scalar(th[:], th[:], 1.0, None, Alu.add)
                nc.vector.tensor_tensor(spkfc[:], vfc[:], th[:], Alu.is_gt)
                mf = wp.tile([128, 32], dt.float16, tag="mf")
                nc.vector.tensor_tensor(mf[:], vfc[:], th[:], Alu.is_le)
                nc.vector.tensor_tensor(vfc[:], vfc[:], mf[:], Alu.mult)
                po = pso.tile([16, N_OUT], dt.float32, tag="po")
                nc.tensor.matmul(po[:], spkfc[:, 0:16], fo[:, 0:N_OUT],
                                 start=True, stop=False)
                nc.tensor.matmul(po[:], spkfc[:, 16:32], fo[:, N_OUT:2 * N_OUT],
                                 start=False, stop=True)
                nc.vector.scalar_tensor_tensor(vo[:], vo[:], float(_BO[0]), po[:],
                                               Alu.mult, Alu.add)
                nc.vector.tensor_tensor(acc[:], acc[:], vo[:], Alu.add)
                yield

            repsq, spk2q = {}, {}
            gens = []
            for it in range(T + 2):
                active = []
                if it < T:
                    active.append(stageA(it))
                if 1 <= it <= T:
                    active.append(stageB(it - 1))
                if 2 <= it:
                    active.append(stageC(it - 2))
                # drain this iteration's stage generators
                if CFG["interleave"]:
                    while active:
                        for g in list(active):
                            if next(g, StopIteration) is StopIteration:
                                active.remove(g)
                else:
                    for g in active:
                        for _ in g:
                            pass

            nc.sync.dma_start(OUT[:], acc[:])
            if CFG.get("debug_out"):
                dv1 = st.tile([32, 3136], dt.float32)
                dv2 = st.tile([64, 400], dt.float32)
                nc.vector.tensor_copy(dv1[:], v1[:, 0:3136])
                nc.vector.tensor_copy(dv2[:], v2[:])
                nc.sync.dma_start(OUTV1[:], dv1[:])
                nc.sync.dma_start(OUTV2[:], dv2[:])

    nc.compile()
    return nc


_BO = [0.0]
_NC_CACHE = None
# tuning knobs (sim experiments)
CFG = {"lif1_split": 8, "x36_pool_ci1": False, "reps_dma": False, "xp_bufs": 4, "ps1_bufs": 2, "evac_dve": False, "interleave": False, "debug_out": False}


def _prep(inputs):
    """Host-side folding of BN/pool/decay constants into packed weights."""
    import ml_dtypes
    f64 = lambda a: np.asarray(a, np.float64)
    s1 = f64(inputs["bn1_gamma"]) / np.sqrt(f64(inputs["bn1_var"]) + EPS)
    sh1 = f64(inputs["bn1_beta"]) - f64(inputs["bn1_mean"]) * s1
    s2 = f64(inputs["bn2_gamma"]) / np.sqrt(f64(inputs["bn2_var"]) + EPS)
    sh2 = f64(inputs["bn2_beta"]) - f64(inputs["bn2_mean"]) * s2
    b1 = _sig(inputs["beta_conv1_raw"])
    b2 = _sig(inputs["beta_conv2_raw"])
    alpha = _sig(inputs["alpha_raw"])
    rho = _sig(inputs["rho_raw"])
    bo = float(_sig(inputs["beta_out"]))

    w1 = f64(inputs["conv1_w"])
    w2 = f64(inputs["conv2_w"])
    w1e = np.zeros((32, 2, 6, 6))
    w2e = np.zeros((64, 32, 6, 6))
    for dy in range(5):
        for dx in range(5):
            w1e[:, :, dy:dy + 2, dx:dx + 2] += 0.25 * w1[:, :, dy, dx][:, :, None, None]
            w2e[:, :, dy:dy + 2, dx:dx + 2] += 0.25 * w2[:, :, dy, dx][:, :, None, None]
    w1e *= (s1 * (1 - b1))[:, None, None, None]
    w2e *= (s2 * (1 - b2))[:, None, None, None]

    # A1 rows p=(e*12+ci*6+j), cols f*32+o = w1e[o, ci, 2e+f, j]
    A1 = np.zeros((36, 64), np.float32)
    for e in range(3):
        for ci in range(2):
            for j in range(6):
                p = e * 12 + ci * 6 + j
                for f in range(2):
                    A1[p, f * 32:(f + 1) * 32] = w1e[:, ci, 2 * e + f, j]
    D1 = np.diag(b1).astype(np.float32)
    B1C = ((1 - b1) * sh1).astype(np.float32).reshape(32, 1)

    # A2 rows (j2g,c), cols (ey*2+jp)*64+o = -w2e[o, c, ey, 2*j2g+jp]
    A2 = np.zeros((96, 12 * 64), np.float32)
    for j2g in range(3):
        for c in range(32):
            for ey in range(6):
                for jp in range(2):
                    A2[j2g * 32 + c, (ey * 2 + jp) * 64:(ey * 2 + jp + 1) * 64] = \
                        -w2e[:, c, ey, 2 * j2g + jp]
    D2 = np.diag(b2).astype(np.float32)
    B2C = ((1 - b2) * sh2 + w2e.sum(axis=(1, 2, 3))).astype(np.float32).reshape(64, 1)

    # F1T rows (sh,c) -> (sh*64+c), cols (s13*2+h)*128+u = fc1w'[128h+u, c*25+2*s13+sh]
    f1 = f64(inputs["fc1_w"]) * (1 - alpha)[:, None]  # (256,1600)
    F1T = np.zeros((128, 26 * 128), np.float32)
    for sh in range(2):
        for s13 in range(13):
            s = 2 * s13 + sh
            if s >= 25:
                continue
            for h in range(2):
                # [64c, 128u]
                F1T[sh * 64:(sh + 1) * 64, (s13 * 2 + h) * 128:(s13 * 2 + h + 1) * 128] = \
                    f1[128 * h:128 * (h + 1), s::25].T
    ALPH = np.stack([alpha[:128], alpha[128:]], axis=1).astype(np.float32)
    RHO2 = np.stack([rho[:128], rho[128:]], axis=1).astype(np.float32)
    RHOC2 = (1.0 - RHO2).astype(np.float32)
    ba = f64(inputs["beta_a"])
    BA2 = np.stack([ba[:128], ba[128:]], axis=1).astype(np.float32)
    foW = f64(inputs["fc_out_w"]) * (1 - bo) / T  # (11,256)
    FO = np.zeros((128, 2 * N_OUT), np.float32)
    FO[:, 0:N_OUT] = foW[:, 0:128].T
    FO[:, N_OUT:2 * N_OUT] = foW[:, 128:256].T

    h16 = lambda a: np.asarray(a, ml_dtypes.float16 if False else np.float16)
    return dict(A1=h16(A1), D1=h16(D1), B1C=B1C, A2=h16(A2), D2=h16(D2), B2C=B2C,
                F1T=h16(F1T), ALPH=ALPH, RHO2=RHO2, RHOC2=RHOC2, BA2=BA2,
                FO=h16(FO)), bo


def _pack_x(x):
    """x (128,50,2,32,32) f32 -> per-core [16, XB] f16 with row padding."""
    x16 = np.asarray(x, np.float16).reshape(128, T, 2, 1024)
    out = np.zeros((128, XB), np.float16)
    v = out[:, :T * XT].reshape(128, T, 2, XROW)
    v[:, :, :, :1024] = x16
    return out


_JIT_CACHE = None


def _get_jit(nc):
    """Cached jitted shard_map executor mirroring run_bass_via_pjrt."""
    global _JIT_CACHE
    if _JIT_CACHE is not None:
        return _JIT_CACHE
    import jax
    import numpy as _np
    from jax.sharding import Mesh, PartitionSpec
    from jax.experimental.shard_map import shard_map
    import concourse.mybir as mybir
    from concourse import bass2jax
    from concourse.bass2jax import _bass_exec_p, partition_id_tensor

    bass2jax.install_neuronx_cc_hook()
    partition_name = nc.partition_id_tensor.name if nc.partition_id_tensor else None
    in_names, out_names, out_avals, zero_outs = [], [], [], []
    for alloc in nc.m.functions[0].allocations:
        if not isinstance(alloc, mybir.MemoryLocationSet):
            continue
        name = alloc.memorylocations[0].name
        if alloc.kind == "ExternalInput":
            if name != partition_name:
                in_names.append(name)
        elif alloc.kind == "ExternalOutput":
            out_names.append(name)
            shape = tuple(alloc.tensor_shape)
            dtype = mybir.dt.np(alloc.dtype)
            out_avals.append(jax.core.ShapedArray(shape, dtype))
            zero_outs.append(_np.zeros(shape, dtype))
    n_params = len(in_names)
    in_names_all = in_names + out_names + ([partition_name] if partition_name else [])

    def _body(*args):
        operands = list(args)
        if partition_name is not None:
            operands.append(partition_id_tensor())
        outs = _bass_exec_p.bind(
            *operands, out_avals=tuple(out_avals), in_names=tuple(in_names_all),
            out_names=tuple(out_names), lowering_input_output_aliases=(),
            sim_require_finite=True, sim_require_nnan=True, nc=nc)
        return tuple(outs)

    devices = jax.devices()[:8]
    mesh = Mesh(_np.asarray(devices), ("core",))
    n_outs = len(out_avals)
    in_specs = (PartitionSpec("core"),) * (n_params + n_outs)
    out_specs = (PartitionSpec("core"),) * n_outs
    donate = tuple(range(n_params, n_params + n_outs))
    sharded = jax.jit(
        shard_map(_body, mesh=mesh, in_specs=in_specs, out_specs=out_specs,
                  check_rep=False),
        donate_argnums=donate, keep_unused=True)
    _JIT_CACHE = (sharded, in_names, out_names, out_avals, zero_outs, mesh)
    return _JIT_CACHE


def _run(inputs, trace=False):
    global _NC_CACHE
    import numpy as _np
    aux, bo = _prep(inputs)
    _BO[0] = bo
    if _NC_CACHE is None:
        _NC_CACHE = _build_nc()
    nc = _NC_CACHE
    xp_ = _pack_x(inputs["x"])
    in_maps = []
    for c in range(8):
        m = dict(aux)
        m["xr"] = _np.ascontiguousarray(xp_[c * B_LOC:(c + 1) * B_LOC])
        in_maps.append(m)
    for _attempt in range(2):
        try:
            sharded, in_names, out_names, out_avals, zero_outs, mesh = _get_jit(nc)
            concat_in = [_np.concatenate([in_maps[c][nm] for c in range(8)], axis=0)
                         for nm in in_names]
            concat_zeros = [_np.zeros((8 * z.shape[0], *z.shape[1:]), z.dtype)
                            for z in zero_outs]
            out_arrs = sharded(*concat_in, *concat_zeros)
            oi = out_names.index("out")
            full = _np.asarray(out_arrs[oi]).reshape(8, B_LOC, N_OUT)
            out = full.reshape(8 * B_LOC, N_OUT)
            return out.astype(_np.float32), None
        except Exception:
            continue
    if True:
        from concourse.bass_utils import run_bass_kernel_spmd
        res = run_bass_kernel_spmd(nc, in_maps, core_ids=list(range(8)),
                                   trace=trace)
        out = _np.concatenate([res.results[c]["out"] for c in range(8)], axis=0)
        return out.astype(_np.float32), res


def kernel(**inputs) -> np.ndarray:
    out, _ = _run(inputs)
    return out

